# revision 13
# baseline (speedup 1.0000x reference)
"""Trainium2 Bass kernel for nn_BlockRevert.

Computation (per batch b, token s):
  out[b,s,0,:]   = temporal_block[b,s,0,:] + pe[s,:] + mod_emb[0,:]
  out[b,s,r+1,:] = (valid[b,s,idx] if idx<8 else mask_token) + pe[s,:] + mod_emb[r+1,:]
     where idx = revert_idx[b,s,r], valid[b,s,j] = temporal_block[b,s,1+j,:]

Sharding: data-parallel over batch, 1 batch per NeuronCore (8 cores).
Per core the gather is fully local.

Everything runs in bf16 (rel-err budget is 2e-2; bf16 end-to-end costs
~5e-3). The gather stage is bound by per-descriptor overhead (~185ns
per SDMA engine per descriptor, mostly HBM random-read latency), not
bytes, so the kernel gathers TWO output slots per descriptor:

The host lays each token's 9 source rows (8 valid + mask) out along a
fixed de Bruijn B(9,2) sequence (82 rows per token), in which every
ordered pair (a,b) of the 9 symbols appears as consecutive rows. Any
pair of adjacent output slots is then one 2-row descriptor
(elem_step=D, elem_size=2*D), halving descriptor count. The layout is
a fixed, input-independent rearrangement (np.take with a constant
index vector); revert_idx only ever feeds index computation, never
data placement.

Gathers are per 128-token block with block-relative indices (max
128*82 = 10496) to stay within the int16 index range. The global slot
(m=0) is one sequential DMA from a contiguous [S, D] tensor.

Device program per core (token-major, 4 blocks of 128 tokens), index
and pe loads hoisted to the front:
  tile t[128 tokens, 17*512 bf16]:
    dma glb block      -> t[:, slot 0]
    dma_gather 4-slot chunks = 2 pair-descriptors per token
    per chunk: += pe (broadcast over slots), += modrep, store
  Each chunk pipelines SDMA -> DVE -> SDMA independently.
"""

import os
import sys

import numpy as np

for _p in ("/opt/trn_rl_repo",):
    if _p not in sys.path and os.path.isdir(_p):
        sys.path.insert(0, _p)

import ml_dtypes

BF16 = ml_dtypes.bfloat16

B, S, MV, D, R = 8, 512, 8, 512, 16
NSLOT = R + 1          # 17 output slots
W = NSLOT * D          # 8704 elems per output row
NSYM = MV + 1          # 9 symbols: valid rows 0..7, mask = 8
NTR = NSYM * NSYM + 1  # 82 de-Bruijn rows per token
NT = S * NTR + 1       # table rows per batch (+1 pad so the last 2-row
                       # window stays in bounds; indices are block-relative)
BLK = 128              # tokens per block
NBLK = S // BLK
NPAIR = R // 2         # 8 slot-pairs per token (slots 1..16)
NIDX = BLK * NPAIR     # 1024 pair-descriptors per block
BROW = BLK * NTR       # table rows per block
GCHUNKS = (1, 5, 9, 13, 17)  # chunk boundaries in slot space

MODE = os.environ.get("BLOCKREVERT_MODE", "pair")


def _de_bruijn_92():
    """B(9,2): seq[82] containing every ordered pair of 0..8 consecutively,
    and pos[a,b] = index i with (seq[i], seq[i+1]) == (a, b)."""
    out = {a: list(range(NSYM - 1, -1, -1)) for a in range(NSYM)}
    stack, circuit = [0], []
    while stack:
        vtx = stack[-1]
        if out[vtx]:
            stack.append(out[vtx].pop())
        else:
            circuit.append(stack.pop())
    seq = np.array(circuit[::-1], dtype=np.int64)
    pos = np.empty((NSYM, NSYM), dtype=np.int64)
    for i in range(NSYM * NSYM):
        pos[seq[i], seq[i + 1]] = i
    return seq, pos


DB_SEQ, DB_POS = _de_bruijn_92()


def _sinusoidal_pe(seq_len, d_model):
    pos = np.arange(seq_len)[:, None].astype(np.float32)
    div = np.exp(
        np.arange(0, d_model, 2).astype(np.float32) * (-np.log(10000.0) / d_model)
    )
    pe = np.zeros((seq_len, d_model), dtype=np.float32)
    pe[:, 0::2] = np.sin(pos * div)
    pe[:, 1::2] = np.cos(pos * div)
    return pe


def build_nc(mode=MODE, n_iter=None, bench_min_io=False, sp=False, coarse=False,
             cbufs=1):
    import concourse.bacc as bacc
    import concourse.mybir as mybir
    import concourse.tile as tile
    from concourse.ap import AP

    bf16 = mybir.dt.bfloat16
    i16 = mybir.dt.int16

    nc = bacc.Bacc("TRN2", target_bir_lowering=False, debug=False)

    # bench_min_io: only gidx crosses the axon tunnel; data tensors are
    # device-resident garbage and the big output stays in DRAM. Timing is
    # unaffected, transfer noise drops ~20x.
    big = "Internal" if bench_min_io else "ExternalInput"
    tbl = nc.dram_tensor("tbl", [NT, D], bf16, kind=big)
    glb = nc.dram_tensor("glb", [S, D], bf16, kind=big)
    # per-block pair-descriptor indices: wrapped into 16 partitions and
    # replicated across the 8 gpsimd cores -> [128, NIDX/16] per block
    gidx = nc.dram_tensor("gidx", [NBLK * BLK, NIDX // 16], i16, kind="ExternalInput")
    pe = nc.dram_tensor("pe", [S, D], bf16, kind=big)
    modrep = nc.dram_tensor("modrep", [BLK, W], bf16, kind=big)
    out = nc.dram_tensor(
        "out", [S, W], bf16, kind="Internal" if bench_min_io else "ExternalOutput"
    )
    tiny = (
        nc.dram_tensor("tiny", [1, 64], i16, kind="ExternalOutput")
        if bench_min_io
        else None
    )

    with tile.TileContext(nc) as tc:
        with (
            tc.tile_pool(name="const", bufs=cbufs) as cpool,
            tc.tile_pool(name="work", bufs=3) as wpool,
            tc.tile_pool(name="small", bufs=3) as spool,
        ):

            def body():
                modt = cpool.tile([BLK, W], bf16)
                nc.sync.dma_start(out=modt[:], in_=modrep.ap())

                # hoist all index/pe loads so gathers start immediately
                its, pts = [], []
                for i in range(NBLK):
                    it = spool.tile([BLK, NIDX // 16], i16, tag=f"it{i}")
                    pt = spool.tile([BLK, D], bf16, tag=f"pt{i}")
                    nc.sync.dma_start(
                        out=it[:], in_=gidx.ap()[i * BLK : (i + 1) * BLK]
                    )
                    nc.sync.dma_start(out=pt[:], in_=pe.ap()[i * BLK : (i + 1) * BLK])
                    its.append(it)
                    pts.append(pt)

                for i in range(NBLK):
                    s0 = i * BLK
                    t = wpool.tile([BLK, W], bf16)
                    it, pt = its[i], pts[i]
                    # this block's 82-row-per-token table slice as an
                    # overlapping-window AP (each index reads rows idx, idx+1);
                    # indices are block-relative so they fit int16
                    base = tbl.ap()
                    tbl_blk = AP(
                        tensor=base.tensor,
                        offset=i * BROW * D,
                        ap=[(D, BROW), (1, 2 * D)],
                    )
                    # global slot: one sequential DMA into slot 0
                    nc.sync.dma_start(out=t[:, 0:D], in_=glb.ap()[s0 : s0 + BLK])

                    def gather(slo, shi):
                        npr = (shi - slo) // 2          # pairs in this span
                        per = npr * BLK                 # descriptors
                        plo = (slo - 1) // 2            # first pair index
                        tv = t[:, slo * D : shi * D].rearrange(
                            "p (m d) -> p m d", d=2 * D
                        )
                        nc.gpsimd.dma_gather(
                            out_ap=tv,
                            in_ap=tbl_blk,
                            idxs_ap=it[
                                :, (plo * BLK) // 16 : ((plo + npr) * BLK) // 16
                            ],
                            num_idxs=per,
                            num_idxs_reg=per,
                            elem_size=2 * D,
                            elem_step=D,
                            single_packet=sp,
                        )

                    if coarse:
                        gather(1, 9)
                        gather(9, 17)
                    for ci in range(len(GCHUNKS) - 1):
                        slo, shi = GCHUNKS[ci], GCHUNKS[ci + 1]
                        if not coarse:
                            gather(slo, shi)
                        # add/store chunk: include slot 0 in the first chunk
                        alo = 0 if ci == 0 else slo
                        ansl = shi - alo
                        av = t[:, alo * D : shi * D].rearrange(
                            "p (m d) -> p m d", d=D
                        )
                        pe_b = pt[:].unsqueeze(1).to_broadcast([BLK, ansl, D])
                        nc.vector.tensor_add(out=av, in0=av, in1=pe_b)
                        nc.vector.tensor_add(
                            out=t[:, alo * D : shi * D],
                            in0=t[:, alo * D : shi * D],
                            in1=modt[:, alo * D : shi * D],
                        )
                        nc.sync.dma_start(
                            out=out.ap()[s0 : s0 + BLK, alo * D : shi * D],
                            in_=t[:, alo * D : shi * D],
                        )

            if n_iter is None:
                body()
            else:
                with tc.For_i(0, n_iter):
                    body()
            if tiny is not None:
                ft = spool.tile([1, 64], i16, tag="fin")
                nc.sync.dma_start(out=ft[:], in_=gidx.ap()[0:1, 0:64])
                nc.sync.dma_start(out=tiny.ap(), in_=ft[:])

    nc.compile()
    return nc


def _wrap_idx(g_blk):
    """[BLK tokens, NPAIR] int16 -> dma_gather wrapped [BLK, NIDX/16]."""
    idxk = g_blk.T.reshape(-1)                # k = pair*128 + p
    w16 = idxk.reshape(NIDX // 16, 16).T      # [16, NIDX/16]
    return np.tile(w16, (8, 1))               # replicate across gpsimd cores


def pair_indices(revert_idx_b):
    """[S, R] revert indices -> block-relative pair-descriptor rows [S, NPAIR]."""
    sym = np.where(revert_idx_b < MV, revert_idx_b, MV)  # 0..7 valid, 8 mask
    a = sym[:, 0::2]
    bsym = sym[:, 1::2]
    srel = (np.arange(S, dtype=np.int64) % BLK) * NTR
    return srel[:, None] + DB_POS[a, bsym]


def make_in_maps(temporal_block, mask_token, mod_emb, revert_idx, mode=MODE):
    temporal_block = np.asarray(temporal_block, dtype=np.float32)
    mask_token = np.asarray(mask_token, dtype=np.float32)
    mod_emb = np.asarray(mod_emb, dtype=np.float32)
    revert_idx = np.asarray(revert_idx).astype(np.int64)

    pe = _sinusoidal_pe(S, D).astype(BF16)
    modrep = np.ascontiguousarray(
        np.broadcast_to(mod_emb[:NSLOT].astype(BF16).reshape(1, W), (BLK, W))
    )

    tb16 = temporal_block.astype(BF16)
    mask16 = mask_token.astype(BF16)
    # token9[s] = [valid rows 0..7, mask]; table = token9[:, DB_SEQ, :]
    tok9 = np.concatenate(
        [tb16[:, :, 1:, :], np.broadcast_to(mask16, (B, S, 1, D))], axis=2
    )  # [B, S, 9, D]
    tbl_all = np.concatenate(
        [
            np.ascontiguousarray(tok9[:, :, DB_SEQ, :]).reshape(B, S * NTR, D),
            np.zeros((B, 1, D), dtype=BF16),
        ],
        axis=1,
    )
    glb_all = np.ascontiguousarray(tb16[:, :, 0, :])  # [B, S, D]

    in_maps = []
    for b in range(B):
        g = pair_indices(revert_idx[b]).astype(np.int16)  # [S, NPAIR]
        gw = np.empty((NBLK, BLK, NIDX // 16), dtype=np.int16)
        for i in range(NBLK):
            gw[i] = _wrap_idx(g[i * BLK : (i + 1) * BLK])
        in_maps.append(
            {
                "tbl": tbl_all[b],
                "glb": glb_all[b],
                "gidx": np.ascontiguousarray(gw.reshape(NBLK * BLK, NIDX // 16)),
                "pe": pe,
                "modrep": modrep,
            }
        )
    return in_maps


def make_bench_arrays(rng, real_gidx=None):
    """Input arrays (one core's worth) for the bench repeat-loop."""
    gidx = real_gidx
    if gidx is None:
        g = rng.integers(0, BROW - NTR, size=(S, NPAIR), dtype=np.int16)
        gw = np.empty((NBLK, BLK, NIDX // 16), dtype=np.int16)
        for i in range(NBLK):
            gw[i] = _wrap_idx(g[i * BLK : (i + 1) * BLK])
        gidx = np.ascontiguousarray(gw.reshape(NBLK * BLK, NIDX // 16))
    return {
        "tbl": rng.standard_normal((NT, D), dtype=np.float32).astype(BF16),
        "glb": rng.standard_normal((S, D), dtype=np.float32).astype(BF16),
        "gidx": gidx,
        "pe": rng.standard_normal((S, D), dtype=np.float32).astype(BF16),
        "modrep": rng.standard_normal((BLK, W), dtype=np.float32).astype(BF16),
    }


_CACHE = {}


def _get_nc(mode=MODE):
    if mode not in _CACHE:
        _CACHE[mode] = build_nc(mode)
    return _CACHE[mode]


def kernel(temporal_block, mask_token, mod_emb, revert_idx):
    from concourse.bass_utils import run_bass_kernel_spmd

    nc = _get_nc()
    in_maps = make_in_maps(temporal_block, mask_token, mod_emb, revert_idx)
    res = run_bass_kernel_spmd(nc, in_maps, core_ids=list(range(B)))
    out = np.stack(
        [
            np.asarray(res.results[b]["out"]).astype(np.float32).reshape(S, NSLOT, D)
            for b in range(B)
        ]
    )
    return out
